# revision 20
# baseline (speedup 1.0000x reference)
"""Trainium2 Bass kernel for a dense transformer block (B=2, T=2048, C=1024, H=16).

Sharding (v2, tensor-parallel attention + ReduceScatter):
  core c -> batch b = c//4, head-group g = c%4 (heads 4g..4g+3),
  own token rows r0 = 512*g of its batch.

Per core:
  LN1 over the full sequence (token-major) -> PE-transpose -> xnT
  (feature-major).  qkv only for the core's 4 heads (q,k feature-major
  via pattern B; v token-major via pattern A with a ones column per head
  for the softmax denominator).  Attention is block-causal EXACT and
  static: every core processes all 2048 queries for its heads, so the
  causal structure is identical across cores; only 4 static staircase
  masks are needed on the diagonal band.  Scores S^T = k^T.T @ q^T (keys
  on partitions), exp on ACT (scale=1/8 folded in), av accumulates the
  denominator in row 64, normalize via reciprocal + gpsimd
  partition_broadcast.  proj partial = y_heads @ Wp[head rows] computed
  token-major (pattern A), then ReduceScatter(add) over the 4-core batch
  group hands each core the reduced rows it owns.  Residual + LN2 + MLP
  (row-sharded, fc/fc2 full) finish locally.

LayerNorm affines are folded into the following matmul weights on the
host (exact).  Matmuls run as float32r (full PE rate at N>=256);
attention q/k/v/P are bf16.
"""

from contextlib import ExitStack

import ml_dtypes
import numpy as np

import concourse.bass as bass
import concourse.tile as tile
import concourse.bacc as bacc
import concourse.mybir as mybir
from concourse.bass_utils import run_bass_kernel_spmd
from concourse.masks import make_identity

F32 = mybir.dt.float32
F32R = mybir.dt.float32r
BF16 = mybir.dt.bfloat16
ALU = mybir.AluOpType
ACTF = mybir.ActivationFunctionType

B, T, C = 2, 2048, 1024
H, DH = 16, 64
FF = 4096
EPS = 1e-5
NCORES = 8
ROWS = 512            # token rows owned per core (MLP phase)
HG = 4                # heads per core
HGF = HG * DH         # 256 head-group features
NTT = T // 128        # 16 token tiles
NOT = ROWS // 128     # 4 own token tiles
NCP = C // 128        # 8 feature partition-tiles of C
NFP = FF // 128       # 32 feature partition-tiles of FF
NQC = T // 512        # 4 query chunks
VSTRIDE = DH + 1      # v stored with a ones column per head


def r(ap, pat, **kw):
    return ap.rearrange(pat, **kw)


def build_program():
    nc = bacc.Bacc("TRN2", target_bir_lowering=False, debug=False,
                   num_devices=NCORES)

    def din(name, shape, dtype=F32):
        return nc.dram_tensor(name, list(shape), dtype, kind="ExternalInput")

    xb = din("xb", (T, C))
    xo_d = din("xo", (ROWS, C))
    cmask = din("cmask", (128, 4 * 512), BF16)
    w_qkv = din("w_qkv", (C, 3 * HGF), BF16)      # q|k|v for this head group
    w_pr = din("w_pr", (HGF, C), F32R)            # proj rows for this group
    w_fc = din("w_fc", (C, FF), F32R)
    w_fc2 = din("w_fc2", (FF, C), F32R)
    b_qk_col = din("b_qk_col", (128, 4))          # q0 q1 k0 k1 bias columns
    b_v_bc = din("b_v_bc", (128, HGF))
    b_proj_bc = din("b_proj_bc", (128, C))
    b_fc_col = din("b_fc_col", (128, 32))
    b_fc2_col = din("b_fc2_col", (128, 8))
    out = nc.dram_tensor("out", [ROWS, C], F32, kind="ExternalOutput")

    with tile.TileContext(nc) as tc, ExitStack() as ctx:
        # ---- constants ----
        cpool = ctx.enter_context(tc.tile_pool(name="const", bufs=1))
        ident = cpool.tile([128, 128], F32, tag="ident")
        make_identity(nc, ident[:])
        bqk = cpool.tile([128, 4], F32, tag="bqk")
        nc.sync.dma_start(bqk[:], b_qk_col.ap())
        bvbc = cpool.tile([128, HGF], F32, tag="bvbc")
        nc.sync.dma_start(bvbc[:], b_v_bc.ap())
        bprbc = cpool.tile([128, C], F32, tag="bprbc")
        nc.sync.dma_start(bprbc[:], b_proj_bc.ap())
        bfc = cpool.tile([128, 32], F32, tag="bfc")
        nc.sync.dma_start(bfc[:], b_fc_col.ap())
        bfc2 = cpool.tile([128, 8], F32, tag="bfc2")
        nc.sync.dma_start(bfc2[:], b_fc2_col.ap())
        mtile = cpool.tile([128, 4 * 512], BF16, tag="mtile")
        nc.sync.dma_start(mtile[:], cmask.ap())
        epsc = cpool.tile([128, 1], F32, tag="epsc")
        nc.gpsimd.memset(epsc[:], EPS)

        def layernorm_apply(spool, xt, xn_out, on_act=False):
            """xn_out = (xt - mean) * rsqrt(var + eps), rowwise over 1024."""
            st = spool.tile([128, 12], F32, tag="st")
            nc.vector.bn_stats(st[:, 0:6], xt[:, 0:512])
            nc.vector.bn_stats(st[:, 6:12], xt[:, 512:1024])
            ag = spool.tile([128, 2], F32, tag="ag")
            nc.vector.bn_aggr(ag[:], r(st, "p (c s) -> p c s", s=6))
            sd = spool.tile([128, 1], F32, tag="sd")
            nc.scalar.activation(sd[:], ag[:, 1:2], ACTF.Sqrt, bias=epsc[:],
                                 scale=1.0)
            rc = spool.tile([128, 1], F32, tag="rc")
            nc.vector.reciprocal(rc[:], sd[:])
            if on_act:
                # xn = x * r + (-mu * r), as one ACT pass (DVE stays free)
                nmr = spool.tile([128, 1], F32, tag="nmr")
                nc.vector.tensor_scalar(nmr[:], ag[:, 0:1], rc[:], -1.0,
                                        op0=ALU.mult, op1=ALU.mult)
                nc.scalar.activation(xn_out, xt, ACTF.Identity,
                                     bias=nmr[:], scale=rc[:])
            else:
                nc.vector.tensor_scalar(xn_out, xt, ag[:, 0:1], rc[:],
                                        op0=ALU.subtract, op1=ALU.mult)

        # yT: attention output, feature-major [2 x (2 heads)][128, T]
        yTpool = ctx.enter_context(tc.tile_pool(name="yT", bufs=1))
        yT = [yTpool.tile([128, T], F32R, tag=f"yT{i}", name=f"yT{i}")
              for i in range(2)]
        # DRAM bounce buffers for the collective
        drpool = ctx.enter_context(tc.tile_pool(name="dram", bufs=1,
                                                space="DRAM"))
        pp_d = drpool.tile([T, C], BF16, tag="pp_d", name="pp_d")
        rs_d = drpool.tile([ROWS, C], BF16, tag="rs_d", name="rs_d")

        with ExitStack() as qctx:
            kvq = qctx.enter_context(tc.tile_pool(name="kvq", bufs=1))
            qTb = [kvq.tile([128, T], BF16, tag=f"qT{i}", name=f"qT{i}")
                   for i in range(2)]
            kTb = [kvq.tile([128, T], BF16, tag=f"kT{i}", name=f"kT{i}")
                   for i in range(2)]
            vb = [kvq.tile([128, HG * VSTRIDE], BF16, tag=f"v{i}", name=f"v{i}")
                  for i in range(NTT)]

            # ============= Phase A: LN1 + transpose to feature-major =======
            with tc.tile_pool(name="phAxnT", bufs=1) as xnTpool:
                xnT = [xnTpool.tile([128, T], BF16, tag=f"xnT{i}",
                                    name=f"xnT{i}") for i in range(NCP)]
                with tc.tile_pool(name="phA", bufs=4) as apool, \
                     tc.tile_pool(name="phAs", bufs=6) as aspool, \
                     tc.tile_pool(name="phAps", bufs=8, space="PSUM") as atps:
                    identb = apool.tile([128, 128], BF16, tag="identb",
                                        bufs=1)
                    nc.vector.tensor_copy(identb[:], ident[:])
                    for tt in range(NTT):
                        xt = apool.tile([128, C], F32, tag="x")
                        nc.sync.dma_start(xt[:],
                                          xb.ap()[tt * 128:(tt + 1) * 128, :])
                        xn = apool.tile([128, C], BF16, tag="xn")
                        layernorm_apply(aspool, xt[:], xn[:], on_act=True)
                        for pt in range(NCP):
                            tp = atps.tile([128, 128], BF16, tag="tp")
                            nc.tensor.transpose(
                                tp[:], xn[:, pt * 128:(pt + 1) * 128],
                                identb[:])
                            nc.vector.tensor_copy(
                                xnT[pt][:, tt * 128:(tt + 1) * 128], tp[:])

                # ============= Phase B: qkv for this head group ============
                with tc.tile_pool(name="wqkv", bufs=1) as wpool, \
                     tc.tile_pool(name="qkps", bufs=3, space="PSUM") as qkps:
                    wq = [wpool.tile([128, 3 * HGF], BF16, tag=f"wq{i}",
                                     name=f"wq{i}") for i in range(NCP)]
                    for kt in range(NCP):
                        nc.sync.dma_start(
                            wq[kt][:], w_qkv.ap()[kt * 128:(kt + 1) * 128, :])
                    # q,k feature-major (pattern B): M-tiles q0 q1 k0 k1
                    for m in range(4):
                        dstl = qTb if m < 2 else kTb
                        dst = dstl[m % 2]
                        for tcix in range(NQC):
                            ps = qkps.tile([128, 512], F32, tag="qk")
                            for kt in range(NCP):
                                nc.tensor.matmul(
                                    ps[:],
                                    wq[kt][:, m * 128:(m + 1) * 128],
                                    xnT[kt][:, tcix * 512:(tcix + 1) * 512],
                                    start=(kt == 0), stop=(kt == NCP - 1))
                            nc.vector.tensor_scalar(
                                dst[:, tcix * 512:(tcix + 1) * 512], ps[:],
                                bqk[:, m:m + 1], None, op0=ALU.add)
                    # v token-major (pattern A) with ones column
                    for tt in range(NTT):
                        nc.gpsimd.memset(
                            r(vb[tt], "p (h m) -> p h m",
                              m=VSTRIDE)[:, :, DH:DH + 1], 1.0)
                        ps = qkps.tile([128, HGF], F32, tag="vp")
                        for kt in range(NCP):
                            nc.tensor.matmul(
                                ps[:],
                                xnT[kt][:, tt * 128:(tt + 1) * 128],
                                wq[kt][:, 2 * HGF:3 * HGF],
                                start=(kt == 0), stop=(kt == NCP - 1))
                        dst = r(vb[tt], "p (h m) -> p h m",
                                m=VSTRIDE)[:, :, 0:DH]
                        nc.vector.tensor_tensor(
                            dst, r(ps[:], "p (h m) -> p h m", m=DH),
                            r(bvbc[:], "p (h m) -> p h m", m=DH), op=ALU.add)

            # ============= Phase C: attention (exact block-causal) =========
            with tc.tile_pool(name="att", bufs=4) as atpool, \
                 tc.tile_pool(name="attsm", bufs=3) as smpool, \
                 tc.tile_pool(name="scps", bufs=3, space="PSUM") as scps, \
                 tc.tile_pool(name="avps", bufs=1, space="PSUM") as avps:
                for pt in range(2):
                    for qc in range(NQC):
                        nkt = 4 * (qc + 1)
                        avs = [avps.tile([128, 512], F32, tag=f"av{s}",
                                         name=f"av_{pt}_{qc}_{s}")
                               for s in range(2)]
                        for kp in range(nkt // 2):
                            for sub in range(2):
                                h = 2 * pt + sub
                                hb = 64 * sub
                                sc = scps.tile([128, 1024], F32, tag="sc")
                                for j in range(2):
                                    kt = 2 * kp + j
                                    nc.tensor.matmul(
                                        sc[:, j * 512:(j + 1) * 512],
                                        kTb[pt][hb:hb + 64,
                                                kt * 128:(kt + 1) * 128],
                                        qTb[pt][hb:hb + 64,
                                                qc * 512:(qc + 1) * 512],
                                        start=True, stop=True)
                                et = atpool.tile([128, 1024], BF16, tag="e")
                                nc.scalar.activation(et[:], sc[:], ACTF.Exp,
                                                     scale=0.125)
                                for j in range(2):
                                    kt = 2 * kp + j
                                    band = kt - 4 * qc
                                    if band >= 0:
                                        pm = atpool.tile([128, 512], BF16,
                                                         tag="p")
                                        nc.vector.tensor_tensor(
                                            pm[:],
                                            et[:, j * 512:(j + 1) * 512],
                                            mtile[:,
                                                  band * 512:(band + 1) * 512],
                                            op=ALU.mult)
                                        rhs_av = pm[:]
                                    else:
                                        rhs_av = et[:, j * 512:(j + 1) * 512]
                                    nc.tensor.matmul(
                                        avs[sub][0:VSTRIDE, :],
                                        vb[kt][:,
                                               h * VSTRIDE:(h + 1) * VSTRIDE],
                                        rhs_av,
                                        start=(kt == 0), stop=(kt == nkt - 1),
                                        skip_group_check=True)
                        for sub in range(2):
                            hb = 64 * sub
                            rr = smpool.tile([1, 512], F32, tag="rr")
                            nc.vector.reciprocal(rr[:],
                                                 avs[sub][DH:DH + 1, :])
                            bc = smpool.tile([64, 512], F32, tag="bc")
                            nc.gpsimd.partition_broadcast(bc[:], rr[:])
                            nc.vector.tensor_tensor(
                                yT[pt][hb:hb + 64, qc * 512:(qc + 1) * 512],
                                avs[sub][0:DH, :], bc[:], op=ALU.mult)

        # ============= Phase D: proj partial (token-major) + RS ============
        with tc.tile_pool(name="phDw", bufs=1) as dwpool, \
             tc.tile_pool(name="phDe", bufs=3) as depool, \
             tc.tile_pool(name="pps", bufs=3, space="PSUM") as pps:
            wp = [dwpool.tile([128, C], F32R, tag=f"wp{i}", name=f"wp{i}")
                  for i in range(2)]
            for i in range(2):
                nc.sync.dma_start(wp[i][:], w_pr.ap()[i * 128:(i + 1) * 128, :])
            for tt in range(NTT):
                pe = depool.tile([128, C], BF16, tag="pe")
                for cc in range(2):
                    ps = pps.tile([128, 512], F32, tag="pj")
                    for i in range(2):
                        nc.tensor.matmul(
                            ps[:], yT[i][:, tt * 128:(tt + 1) * 128],
                            wp[i][:, cc * 512:(cc + 1) * 512],
                            start=(i == 0), stop=(i == 1))
                    nc.vector.tensor_copy(pe[:, cc * 512:(cc + 1) * 512], ps[:])
                nc.sync.dma_start(pp_d[tt * 128:(tt + 1) * 128, :], pe[:])
            nc.gpsimd.collective_compute(
                "ReduceScatter", ALU.add,
                replica_groups=[[0, 1, 2, 3], [4, 5, 6, 7]],
                ins=[pp_d.opt()], outs=[rs_d.opt()])

        # ============= Phase E: residual + LN2 =============================
        dxpool = ctx.enter_context(tc.tile_pool(name="phDx", bufs=1))
        x2 = [dxpool.tile([128, C], F32, tag=f"x2{i}", name=f"x2{i}")
              for i in range(NOT)]
        xn2T = [dxpool.tile([128, ROWS], F32R, tag=f"xn2T{i}", name=f"xn2T{i}")
                for i in range(NCP)]
        with tc.tile_pool(name="phE", bufs=3) as epool, \
             tc.tile_pool(name="phEs", bufs=4) as espool, \
             tc.tile_pool(name="tps", bufs=4, space="PSUM") as tps:
            for j in range(NOT):
                rs_sb = epool.tile([128, C], BF16, tag="rs")
                nc.sync.dma_start(rs_sb[:], rs_d[j * 128:(j + 1) * 128, :])
                xot = epool.tile([128, C], F32, tag="xot")
                nc.sync.dma_start(xot[:], xo_d.ap()[j * 128:(j + 1) * 128, :])
                xr = epool.tile([128, C], F32, tag="xr")
                nc.vector.tensor_tensor(xr[:], rs_sb[:], bprbc[:], op=ALU.add)
                nc.vector.tensor_tensor(x2[j][:], xr[:], xot[:], op=ALU.add)
                xn2 = epool.tile([128, C], F32, tag="xn2")
                layernorm_apply(espool, x2[j][:], xn2[:])
                for pt in range(NCP):
                    tp = tps.tile([128, 128], F32, tag="tp")
                    nc.tensor.transpose(tp[:], xn2[:, pt * 128:(pt + 1) * 128],
                                        ident[:])
                    nc.vector.tensor_copy(xn2T[pt][:, j * 128:(j + 1) * 128],
                                          tp[:])

        # ============= Phase F: MLP + out ==================================
        with tc.tile_pool(name="phF", bufs=2) as fpool, \
             tc.tile_pool(name="phFh", bufs=1) as hpool, \
             tc.tile_pool(name="fps", bufs=2, space="PSUM") as fps, \
             tc.tile_pool(name="tps2", bufs=4, space="PSUM") as tps2:
            hgT = [hpool.tile([128, ROWS], F32R, tag=f"hg{i}", name=f"hg{i}")
                   for i in range(NFP)]
            outsb = [hpool.tile([128, C], F32, tag=f"os{i}", name=f"os{i}")
                     for i in range(NOT)]
            for m in range(NFP):
                wt = fpool.tile([128, C], F32R, tag="wf")
                nc.sync.dma_start(
                    r(wt[:], "p (k c) -> p k c", c=128),
                    r(w_fc.ap()[:, m * 128:(m + 1) * 128],
                      "(k p) c -> p k c", p=128))
                ps = fps.tile([128, 512], F32, tag="fc")
                for kt in range(NCP):
                    nc.tensor.matmul(
                        ps[:], wt[:, kt * 128:(kt + 1) * 128], xn2T[kt][:],
                        start=(kt == 0), stop=(kt == NCP - 1))
                nc.scalar.activation(hgT[m][:], ps[:], ACTF.Gelu,
                                     bias=bfc[:, m:m + 1], scale=1.0)
            for m2 in range(8):
                wt = fpool.tile([128, FF], F32R, tag="wf2", bufs=2)
                nc.sync.dma_start(
                    r(wt[:], "p (k c) -> p k c", c=128),
                    r(w_fc2.ap()[:, m2 * 128:(m2 + 1) * 128],
                      "(k p) c -> p k c", p=128))
                ps = fps.tile([128, 512], F32, tag="fc")
                for kt2 in range(NFP):
                    nc.tensor.matmul(
                        ps[:], wt[:, kt2 * 128:(kt2 + 1) * 128], hgT[kt2][:],
                        start=(kt2 == 0), stop=(kt2 == NFP - 1))
                y2 = fpool.tile([128, 512], F32, tag="y2")
                nc.vector.tensor_scalar(y2[:], ps[:], bfc2[:, m2:m2 + 1], None,
                                        op0=ALU.add)
                for j in range(NOT):
                    tp = tps2.tile([128, 128], F32, tag="tp")
                    nc.tensor.transpose(tp[:], y2[:, j * 128:(j + 1) * 128],
                                        ident[:])
                    nc.vector.tensor_tensor(
                        outsb[j][:, m2 * 128:(m2 + 1) * 128], tp[:],
                        x2[j][:, m2 * 128:(m2 + 1) * 128], op=ALU.add)
            for j in range(NOT):
                nc.sync.dma_start(out.ap()[j * 128:(j + 1) * 128, :],
                                  outsb[j][:])

    nc.compile()
    return nc


_NC_CACHE = None


def _get_program():
    global _NC_CACHE
    if _NC_CACHE is None:
        _NC_CACHE = build_program()
    return _NC_CACHE


def _prepare_in_maps(x, ln1_g, ln1_b, w_attn, b_attn, w_proj, b_proj,
                     ln2_g, ln2_b, w_fc, b_fc, w_fc2, b_fc2):
    x = np.asarray(x, np.float32)
    ln1_g = np.asarray(ln1_g, np.float32); ln1_b = np.asarray(ln1_b, np.float32)
    w_attn = np.asarray(w_attn, np.float32); b_attn = np.asarray(b_attn, np.float32)
    w_proj = np.asarray(w_proj, np.float32); b_proj = np.asarray(b_proj, np.float32)
    ln2_g = np.asarray(ln2_g, np.float32); ln2_b = np.asarray(ln2_b, np.float32)
    w_fc = np.asarray(w_fc, np.float32); b_fc = np.asarray(b_fc, np.float32)
    w_fc2 = np.asarray(w_fc2, np.float32); b_fc2 = np.asarray(b_fc2, np.float32)

    # Fold LayerNorm affine params into the following matmuls (exact).
    w_attn_f = ln1_g[:, None] * w_attn
    b_attn_f = b_attn + ln1_b @ w_attn
    w_fc_f = ln2_g[:, None] * w_fc
    b_fc_f = b_fc + ln2_b @ w_fc

    # 4 static staircase masks for the diagonal band:
    # mask_i[p, j] = 1 iff j >= 128*i + p (query col j attends band-tile row p)
    jj = np.arange(512)[None, :]
    pp = np.arange(128)[:, None]
    cmask = np.concatenate(
        [(jj >= 128 * i + pp) for i in range(4)],
        axis=1).astype(ml_dtypes.bfloat16)

    shared = {
        "cmask": cmask,
        "w_fc": w_fc_f,
        "w_fc2": w_fc2,
        "b_proj_bc": np.ascontiguousarray(np.broadcast_to(b_proj, (128, C))),
        "b_fc_col": np.ascontiguousarray(b_fc_f.reshape(32, 128).T),
        "b_fc2_col": np.ascontiguousarray(b_fc2.reshape(8, 128).T),
    }

    in_maps = []
    for c in range(NCORES):
        bidx = c // 4
        g = c % 4
        r0 = g * ROWS
        fsl = slice(g * HGF, (g + 1) * HGF)
        w_q = w_attn_f[:, 0 * C:1 * C][:, fsl]
        w_k = w_attn_f[:, 1 * C:2 * C][:, fsl]
        w_v = w_attn_f[:, 2 * C:3 * C][:, fsl]
        b_q = b_attn_f[0 * C:1 * C][fsl]
        b_k = b_attn_f[1 * C:2 * C][fsl]
        b_v = b_attn_f[2 * C:3 * C][fsl]
        m = dict(shared)
        m["xb"] = np.ascontiguousarray(x[bidx])
        m["xo"] = np.ascontiguousarray(x[bidx][r0:r0 + ROWS])
        m["w_qkv"] = np.ascontiguousarray(
            np.concatenate([w_q, w_k, w_v], axis=1)).astype(ml_dtypes.bfloat16)
        m["w_pr"] = np.ascontiguousarray(w_proj[fsl, :])
        m["b_qk_col"] = np.ascontiguousarray(
            np.concatenate([b_q, b_k]).reshape(4, 128).T)
        m["b_v_bc"] = np.ascontiguousarray(np.broadcast_to(b_v, (128, HGF)))
        in_maps.append(m)
    return in_maps


def _gather(res):
    y = np.empty((B, T, C), np.float32)
    for c in range(NCORES):
        bidx = c // 4
        r0 = (c % 4) * ROWS
        y[bidx, r0:r0 + ROWS] = res.results[c]["out"]
    return y


def kernel(**inputs):
    in_maps = _prepare_in_maps(**inputs)
    nc = _get_program()
    res = run_bass_kernel_spmd(nc, in_maps, core_ids=list(range(NCORES)))
    return _gather(res)


def run_traced(inputs, **kw):
    """Run with NTFF tracing; returns (output, BassKernelResults)."""
    in_maps = _prepare_in_maps(**inputs)
    nc = _get_program()
    res = run_bass_kernel_spmd(nc, in_maps, core_ids=list(range(NCORES)),
                               trace=True, **kw)
    return _gather(res), res


# revision 23
# speedup vs baseline: 1.0025x; 1.0025x over previous
"""Trainium2 Bass kernel for a dense transformer block (B=2, T=2048, C=1024, H=16).

Sharding (v2, tensor-parallel attention + ReduceScatter):
  core c -> batch b = c//4, head-group g = c%4 (heads 4g..4g+3),
  own token rows r0 = 512*g of its batch.

Per core:
  LN1 over the full sequence (token-major) -> PE-transpose -> xnT
  (feature-major).  qkv only for the core's 4 heads (q,k feature-major
  via pattern B; v token-major via pattern A with a ones column per head
  for the softmax denominator).  Attention is block-causal EXACT and
  static: every core processes all 2048 queries for its heads, so the
  causal structure is identical across cores; only 4 static staircase
  masks are needed on the diagonal band.  Scores S^T = k^T.T @ q^T (keys
  on partitions), exp on ACT (scale=1/8 folded in), av accumulates the
  denominator in row 64, normalize via reciprocal + gpsimd
  partition_broadcast.  proj partial = y_heads @ Wp[head rows] computed
  token-major (pattern A), then ReduceScatter(add) over the 4-core batch
  group hands each core the reduced rows it owns.  Residual + LN2 + MLP
  (row-sharded, fc/fc2 full) finish locally.

LayerNorm affines are folded into the following matmul weights on the
host (exact).  Matmuls run as float32r (full PE rate at N>=256);
attention q/k/v/P are bf16.
"""

from contextlib import ExitStack

import ml_dtypes
import numpy as np

import concourse.bass as bass
import concourse.tile as tile
import concourse.bacc as bacc
import concourse.mybir as mybir
from concourse.bass_utils import run_bass_kernel_spmd
from concourse.masks import make_identity

F32 = mybir.dt.float32
F32R = mybir.dt.float32r
BF16 = mybir.dt.bfloat16
ALU = mybir.AluOpType
ACTF = mybir.ActivationFunctionType

B, T, C = 2, 2048, 1024
H, DH = 16, 64
FF = 4096
EPS = 1e-5
NCORES = 8
ROWS = 512            # token rows owned per core (MLP phase)
HG = 4                # heads per core
HGF = HG * DH         # 256 head-group features
NTT = T // 128        # 16 token tiles
NOT = ROWS // 128     # 4 own token tiles
NCP = C // 128        # 8 feature partition-tiles of C
NFP = FF // 128       # 32 feature partition-tiles of FF
NQC = T // 512        # 4 query chunks
VSTRIDE = DH + 1      # v stored with a ones column per head


def r(ap, pat, **kw):
    return ap.rearrange(pat, **kw)


def build_program():
    nc = bacc.Bacc("TRN2", target_bir_lowering=False, debug=False,
                   num_devices=NCORES)

    def din(name, shape, dtype=F32):
        return nc.dram_tensor(name, list(shape), dtype, kind="ExternalInput")

    xb = din("xb", (T, C))
    xo_d = din("xo", (ROWS, C))
    cmask = din("cmask", (128, 4 * 512), BF16)
    w_qkv = din("w_qkv", (C, 3 * HGF), BF16)      # q|k|v for this head group
    w_pr = din("w_pr", (HGF, C), F32R)            # proj rows for this group
    w_fc = din("w_fc", (C, FF), F32R)
    w_fc2 = din("w_fc2", (FF, C), F32R)
    b_qk_col = din("b_qk_col", (128, 4))          # q0 q1 k0 k1 bias columns
    b_v_bc = din("b_v_bc", (128, HGF))
    b_proj_bc = din("b_proj_bc", (128, C))
    b_fc_col = din("b_fc_col", (128, 32))
    b_fc2_col = din("b_fc2_col", (128, 8))
    out = nc.dram_tensor("out", [ROWS, C], F32, kind="ExternalOutput")

    with tile.TileContext(nc) as tc, ExitStack() as ctx:
        # ---- constants ----
        cpool = ctx.enter_context(tc.tile_pool(name="const", bufs=1))
        ident = cpool.tile([128, 128], F32, tag="ident")
        make_identity(nc, ident[:])
        bqk = cpool.tile([128, 4], F32, tag="bqk")
        nc.sync.dma_start(bqk[:], b_qk_col.ap())
        bvbc = cpool.tile([128, HGF], F32, tag="bvbc")
        nc.sync.dma_start(bvbc[:], b_v_bc.ap())
        bprbc = cpool.tile([128, C], F32, tag="bprbc")
        nc.sync.dma_start(bprbc[:], b_proj_bc.ap())
        bfc = cpool.tile([128, 32], F32, tag="bfc")
        nc.sync.dma_start(bfc[:], b_fc_col.ap())
        bfc2 = cpool.tile([128, 8], F32, tag="bfc2")
        nc.sync.dma_start(bfc2[:], b_fc2_col.ap())
        mtile = cpool.tile([128, 4 * 512], BF16, tag="mtile")
        nc.sync.dma_start(mtile[:], cmask.ap())
        epsc = cpool.tile([128, 1], F32, tag="epsc")
        nc.gpsimd.memset(epsc[:], EPS)

        def layernorm_apply(spool, xt, xn_out, on_act=False):
            """xn_out = (xt - mean) * rsqrt(var + eps), rowwise over 1024."""
            st = spool.tile([128, 12], F32, tag="st")
            nc.vector.bn_stats(st[:, 0:6], xt[:, 0:512])
            nc.vector.bn_stats(st[:, 6:12], xt[:, 512:1024])
            ag = spool.tile([128, 2], F32, tag="ag")
            nc.vector.bn_aggr(ag[:], r(st, "p (c s) -> p c s", s=6))
            sd = spool.tile([128, 1], F32, tag="sd")
            nc.scalar.activation(sd[:], ag[:, 1:2], ACTF.Sqrt, bias=epsc[:],
                                 scale=1.0)
            rc = spool.tile([128, 1], F32, tag="rc")
            nc.vector.reciprocal(rc[:], sd[:])
            if on_act:
                # xn = x * r + (-mu * r), as one ACT pass (DVE stays free)
                nmr = spool.tile([128, 1], F32, tag="nmr")
                nc.vector.tensor_scalar(nmr[:], ag[:, 0:1], rc[:], -1.0,
                                        op0=ALU.mult, op1=ALU.mult)
                nc.scalar.activation(xn_out, xt, ACTF.Identity,
                                     bias=nmr[:], scale=rc[:])
            else:
                nc.vector.tensor_scalar(xn_out, xt, ag[:, 0:1], rc[:],
                                        op0=ALU.subtract, op1=ALU.mult)

        # yT: attention output, feature-major [2 x (2 heads)][128, T]
        yTpool = ctx.enter_context(tc.tile_pool(name="yT", bufs=1))
        yT = [yTpool.tile([128, T], F32R, tag=f"yT{i}", name=f"yT{i}")
              for i in range(2)]
        # DRAM bounce buffers for the collective
        drpool = ctx.enter_context(tc.tile_pool(name="dram", bufs=1,
                                                space="DRAM"))
        pp_d = drpool.tile([T, C], BF16, tag="pp_d", name="pp_d")
        rs_d = drpool.tile([ROWS, C], BF16, tag="rs_d", name="rs_d")

        with ExitStack() as qctx:
            kvq = qctx.enter_context(tc.tile_pool(name="kvq", bufs=1))
            qTb = [kvq.tile([128, T], BF16, tag=f"qT{i}", name=f"qT{i}")
                   for i in range(2)]
            kTb = [kvq.tile([128, T], BF16, tag=f"kT{i}", name=f"kT{i}")
                   for i in range(2)]
            vb = [kvq.tile([128, HG * VSTRIDE], BF16, tag=f"v{i}", name=f"v{i}")
                  for i in range(NTT)]

            # ============= Phase A: LN1 + transpose to feature-major =======
            with tc.tile_pool(name="phAxnT", bufs=1) as xnTpool:
                xnT = [xnTpool.tile([128, T], BF16, tag=f"xnT{i}",
                                    name=f"xnT{i}") for i in range(NCP)]
                with tc.tile_pool(name="phA", bufs=4) as apool, \
                     tc.tile_pool(name="phAs", bufs=6) as aspool, \
                     tc.tile_pool(name="phAps", bufs=8, space="PSUM") as atps:
                    identb = apool.tile([128, 128], BF16, tag="identb",
                                        bufs=1)
                    nc.vector.tensor_copy(identb[:], ident[:])
                    for tt in range(NTT):
                        xt = apool.tile([128, C], F32, tag="x")
                        nc.sync.dma_start(xt[:],
                                          xb.ap()[tt * 128:(tt + 1) * 128, :])
                        xn = apool.tile([128, C], BF16, tag="xn")
                        layernorm_apply(aspool, xt[:], xn[:], on_act=True)
                        for pt in range(NCP):
                            tp = atps.tile([128, 128], BF16, tag="tp",
                                           name=f"tp_{tt}_{pt}")
                            nc.tensor.transpose(
                                tp[:], xn[:, pt * 128:(pt + 1) * 128],
                                identb[:])
                            if pt < 2:
                                nc.scalar.copy(
                                    xnT[pt][:, tt * 128:(tt + 1) * 128], tp[:])
                            else:
                                nc.vector.tensor_copy(
                                    xnT[pt][:, tt * 128:(tt + 1) * 128], tp[:])

                # ============= Phase B: qkv for this head group ============
                with tc.tile_pool(name="wqkv", bufs=1) as wpool, \
                     tc.tile_pool(name="qkps", bufs=3, space="PSUM") as qkps:
                    wq = [wpool.tile([128, 3 * HGF], BF16, tag=f"wq{i}",
                                     name=f"wq{i}") for i in range(NCP)]
                    for kt in range(NCP):
                        nc.sync.dma_start(
                            wq[kt][:], w_qkv.ap()[kt * 128:(kt + 1) * 128, :])
                    # q,k feature-major (pattern B): M-tiles q0 q1 k0 k1
                    for m in range(4):
                        dstl = qTb if m < 2 else kTb
                        dst = dstl[m % 2]
                        for tcix in range(NQC):
                            ps = qkps.tile([128, 512], F32, tag="qk")
                            for kt in range(NCP):
                                nc.tensor.matmul(
                                    ps[:],
                                    wq[kt][:, m * 128:(m + 1) * 128],
                                    xnT[kt][:, tcix * 512:(tcix + 1) * 512],
                                    start=(kt == 0), stop=(kt == NCP - 1))
                            nc.vector.tensor_scalar(
                                dst[:, tcix * 512:(tcix + 1) * 512], ps[:],
                                bqk[:, m:m + 1], None, op0=ALU.add)
                    # v token-major (pattern A) with ones column
                    for tt in range(NTT):
                        nc.gpsimd.memset(
                            r(vb[tt], "p (h m) -> p h m",
                              m=VSTRIDE)[:, :, DH:DH + 1], 1.0)
                        ps = qkps.tile([128, HGF], F32, tag="vp")
                        for kt in range(NCP):
                            nc.tensor.matmul(
                                ps[:],
                                xnT[kt][:, tt * 128:(tt + 1) * 128],
                                wq[kt][:, 2 * HGF:3 * HGF],
                                start=(kt == 0), stop=(kt == NCP - 1))
                        dst = r(vb[tt], "p (h m) -> p h m",
                                m=VSTRIDE)[:, :, 0:DH]
                        nc.vector.tensor_tensor(
                            dst, r(ps[:], "p (h m) -> p h m", m=DH),
                            r(bvbc[:], "p (h m) -> p h m", m=DH), op=ALU.add)

            # ============= Phase C: attention (exact block-causal) =========
            with tc.tile_pool(name="att", bufs=4) as atpool, \
                 tc.tile_pool(name="attsm", bufs=3) as smpool, \
                 tc.tile_pool(name="scps", bufs=2, space="PSUM") as scps, \
                 tc.tile_pool(name="avps", bufs=1, space="PSUM") as avps:
                for pt in range(2):
                    for qc in range(NQC):
                        nkt = 4 * (qc + 1)
                        avs = [avps.tile([128, 512], F32,
                                         tag=f"av{qc % 2}{s}",
                                         name=f"av_{pt}_{qc}_{s}")
                               for s in range(2)]
                        for kp in range(nkt // 2):
                            for sub in range(2):
                                h = 2 * pt + sub
                                hb = 64 * sub
                                sc = scps.tile([128, 1024], F32, tag="sc")
                                for j in range(2):
                                    kt = 2 * kp + j
                                    nc.tensor.matmul(
                                        sc[:, j * 512:(j + 1) * 512],
                                        kTb[pt][hb:hb + 64,
                                                kt * 128:(kt + 1) * 128],
                                        qTb[pt][hb:hb + 64,
                                                qc * 512:(qc + 1) * 512],
                                        start=True, stop=True)
                                et = atpool.tile([128, 1024], BF16, tag="e")
                                nc.scalar.activation(et[:], sc[:], ACTF.Exp,
                                                     scale=0.125)
                                for j in range(2):
                                    kt = 2 * kp + j
                                    band = kt - 4 * qc
                                    if band >= 0:
                                        pm = atpool.tile([128, 512], BF16,
                                                         tag="p")
                                        nc.vector.tensor_tensor(
                                            pm[:],
                                            et[:, j * 512:(j + 1) * 512],
                                            mtile[:,
                                                  band * 512:(band + 1) * 512],
                                            op=ALU.mult)
                                        rhs_av = pm[:]
                                    else:
                                        rhs_av = et[:, j * 512:(j + 1) * 512]
                                    nc.tensor.matmul(
                                        avs[sub][0:VSTRIDE, :],
                                        vb[kt][:,
                                               h * VSTRIDE:(h + 1) * VSTRIDE],
                                        rhs_av,
                                        start=(kt == 0), stop=(kt == nkt - 1),
                                        skip_group_check=True)
                        for sub in range(2):
                            hb = 64 * sub
                            rr = smpool.tile([1, 512], F32, tag="rr")
                            nc.vector.reciprocal(rr[:],
                                                 avs[sub][DH:DH + 1, :])
                            bc = smpool.tile([64, 512], F32, tag="bc")
                            nc.gpsimd.partition_broadcast(bc[:], rr[:])
                            nc.vector.tensor_tensor(
                                yT[pt][hb:hb + 64, qc * 512:(qc + 1) * 512],
                                avs[sub][0:DH, :], bc[:], op=ALU.mult)

        # ============= Phase D: proj partial (token-major) + RS ============
        with tc.tile_pool(name="phDw", bufs=1) as dwpool, \
             tc.tile_pool(name="phDe", bufs=3) as depool, \
             tc.tile_pool(name="pps", bufs=3, space="PSUM") as pps:
            wp = [dwpool.tile([128, C], F32R, tag=f"wp{i}", name=f"wp{i}")
                  for i in range(2)]
            for i in range(2):
                nc.sync.dma_start(wp[i][:], w_pr.ap()[i * 128:(i + 1) * 128, :])
            for tt in range(NTT):
                pe = depool.tile([128, C], BF16, tag="pe")
                for cc in range(2):
                    ps = pps.tile([128, 512], F32, tag="pj")
                    for i in range(2):
                        nc.tensor.matmul(
                            ps[:], yT[i][:, tt * 128:(tt + 1) * 128],
                            wp[i][:, cc * 512:(cc + 1) * 512],
                            start=(i == 0), stop=(i == 1))
                    nc.vector.tensor_copy(pe[:, cc * 512:(cc + 1) * 512], ps[:])
                nc.sync.dma_start(pp_d[tt * 128:(tt + 1) * 128, :], pe[:])
            nc.gpsimd.collective_compute(
                "ReduceScatter", ALU.add,
                replica_groups=[[0, 1, 2, 3], [4, 5, 6, 7]],
                ins=[pp_d.opt()], outs=[rs_d.opt()])

        # ============= Phase E: residual + LN2 =============================
        dxpool = ctx.enter_context(tc.tile_pool(name="phDx", bufs=1))
        x2 = [dxpool.tile([128, C], F32, tag=f"x2{i}", name=f"x2{i}")
              for i in range(NOT)]
        xn2T = [dxpool.tile([128, ROWS], F32R, tag=f"xn2T{i}", name=f"xn2T{i}")
                for i in range(NCP)]
        with tc.tile_pool(name="phE", bufs=3) as epool, \
             tc.tile_pool(name="phEs", bufs=4) as espool, \
             tc.tile_pool(name="tps", bufs=4, space="PSUM") as tps:
            for j in range(NOT):
                rs_sb = epool.tile([128, C], BF16, tag="rs")
                nc.sync.dma_start(rs_sb[:], rs_d[j * 128:(j + 1) * 128, :])
                xot = epool.tile([128, C], F32, tag="xot")
                nc.sync.dma_start(xot[:], xo_d.ap()[j * 128:(j + 1) * 128, :])
                xr = epool.tile([128, C], F32, tag="xr")
                nc.vector.tensor_tensor(xr[:], rs_sb[:], bprbc[:], op=ALU.add)
                nc.vector.tensor_tensor(x2[j][:], xr[:], xot[:], op=ALU.add)
                xn2 = epool.tile([128, C], F32, tag="xn2")
                layernorm_apply(espool, x2[j][:], xn2[:])
                for pt in range(NCP):
                    tp = tps.tile([128, 128], F32, tag="tp")
                    nc.tensor.transpose(tp[:], xn2[:, pt * 128:(pt + 1) * 128],
                                        ident[:])
                    nc.vector.tensor_copy(xn2T[pt][:, j * 128:(j + 1) * 128],
                                          tp[:])

        # ============= Phase F: MLP + out ==================================
        with tc.tile_pool(name="phF", bufs=2) as fpool, \
             tc.tile_pool(name="phFh", bufs=1) as hpool, \
             tc.tile_pool(name="fps", bufs=2, space="PSUM") as fps, \
             tc.tile_pool(name="tps2", bufs=4, space="PSUM") as tps2:
            hgT = [hpool.tile([128, ROWS], F32R, tag=f"hg{i}", name=f"hg{i}")
                   for i in range(NFP)]
            outsb = [hpool.tile([128, C], F32, tag=f"os{i}", name=f"os{i}")
                     for i in range(NOT)]
            for m in range(NFP):
                wt = fpool.tile([128, C], F32R, tag="wf")
                nc.sync.dma_start(
                    r(wt[:], "p (k c) -> p k c", c=128),
                    r(w_fc.ap()[:, m * 128:(m + 1) * 128],
                      "(k p) c -> p k c", p=128))
                ps = fps.tile([128, 512], F32, tag="fc")
                for kt in range(NCP):
                    nc.tensor.matmul(
                        ps[:], wt[:, kt * 128:(kt + 1) * 128], xn2T[kt][:],
                        start=(kt == 0), stop=(kt == NCP - 1))
                nc.scalar.activation(hgT[m][:], ps[:], ACTF.Gelu,
                                     bias=bfc[:, m:m + 1], scale=1.0)
            for m2 in range(8):
                wt = fpool.tile([128, FF], F32R, tag="wf2", bufs=2)
                nc.sync.dma_start(
                    r(wt[:], "p (k c) -> p k c", c=128),
                    r(w_fc2.ap()[:, m2 * 128:(m2 + 1) * 128],
                      "(k p) c -> p k c", p=128))
                ps = fps.tile([128, 512], F32, tag="fc")
                for kt2 in range(NFP):
                    nc.tensor.matmul(
                        ps[:], wt[:, kt2 * 128:(kt2 + 1) * 128], hgT[kt2][:],
                        start=(kt2 == 0), stop=(kt2 == NFP - 1))
                y2 = fpool.tile([128, 512], F32, tag="y2")
                nc.vector.tensor_scalar(y2[:], ps[:], bfc2[:, m2:m2 + 1], None,
                                        op0=ALU.add)
                for j in range(NOT):
                    tp = tps2.tile([128, 128], F32, tag="tp")
                    nc.tensor.transpose(tp[:], y2[:, j * 128:(j + 1) * 128],
                                        ident[:])
                    nc.vector.tensor_tensor(
                        outsb[j][:, m2 * 128:(m2 + 1) * 128], tp[:],
                        x2[j][:, m2 * 128:(m2 + 1) * 128], op=ALU.add)
            for j in range(NOT):
                nc.sync.dma_start(out.ap()[j * 128:(j + 1) * 128, :],
                                  outsb[j][:])

    nc.compile()
    return nc


_NC_CACHE = None


def _get_program():
    global _NC_CACHE
    if _NC_CACHE is None:
        _NC_CACHE = build_program()
    return _NC_CACHE


def _prepare_in_maps(x, ln1_g, ln1_b, w_attn, b_attn, w_proj, b_proj,
                     ln2_g, ln2_b, w_fc, b_fc, w_fc2, b_fc2):
    x = np.asarray(x, np.float32)
    ln1_g = np.asarray(ln1_g, np.float32); ln1_b = np.asarray(ln1_b, np.float32)
    w_attn = np.asarray(w_attn, np.float32); b_attn = np.asarray(b_attn, np.float32)
    w_proj = np.asarray(w_proj, np.float32); b_proj = np.asarray(b_proj, np.float32)
    ln2_g = np.asarray(ln2_g, np.float32); ln2_b = np.asarray(ln2_b, np.float32)
    w_fc = np.asarray(w_fc, np.float32); b_fc = np.asarray(b_fc, np.float32)
    w_fc2 = np.asarray(w_fc2, np.float32); b_fc2 = np.asarray(b_fc2, np.float32)

    # Fold LayerNorm affine params into the following matmuls (exact).
    w_attn_f = ln1_g[:, None] * w_attn
    b_attn_f = b_attn + ln1_b @ w_attn
    w_fc_f = ln2_g[:, None] * w_fc
    b_fc_f = b_fc + ln2_b @ w_fc

    # 4 static staircase masks for the diagonal band:
    # mask_i[p, j] = 1 iff j >= 128*i + p (query col j attends band-tile row p)
    jj = np.arange(512)[None, :]
    pp = np.arange(128)[:, None]
    cmask = np.concatenate(
        [(jj >= 128 * i + pp) for i in range(4)],
        axis=1).astype(ml_dtypes.bfloat16)

    shared = {
        "cmask": cmask,
        "w_fc": w_fc_f,
        "w_fc2": w_fc2,
        "b_proj_bc": np.ascontiguousarray(np.broadcast_to(b_proj, (128, C))),
        "b_fc_col": np.ascontiguousarray(b_fc_f.reshape(32, 128).T),
        "b_fc2_col": np.ascontiguousarray(b_fc2.reshape(8, 128).T),
    }

    in_maps = []
    for c in range(NCORES):
        bidx = c // 4
        g = c % 4
        r0 = g * ROWS
        fsl = slice(g * HGF, (g + 1) * HGF)
        w_q = w_attn_f[:, 0 * C:1 * C][:, fsl]
        w_k = w_attn_f[:, 1 * C:2 * C][:, fsl]
        w_v = w_attn_f[:, 2 * C:3 * C][:, fsl]
        b_q = b_attn_f[0 * C:1 * C][fsl]
        b_k = b_attn_f[1 * C:2 * C][fsl]
        b_v = b_attn_f[2 * C:3 * C][fsl]
        m = dict(shared)
        m["xb"] = np.ascontiguousarray(x[bidx])
        m["xo"] = np.ascontiguousarray(x[bidx][r0:r0 + ROWS])
        m["w_qkv"] = np.ascontiguousarray(
            np.concatenate([w_q, w_k, w_v], axis=1)).astype(ml_dtypes.bfloat16)
        m["w_pr"] = np.ascontiguousarray(w_proj[fsl, :])
        m["b_qk_col"] = np.ascontiguousarray(
            np.concatenate([b_q, b_k]).reshape(4, 128).T)
        m["b_v_bc"] = np.ascontiguousarray(np.broadcast_to(b_v, (128, HGF)))
        in_maps.append(m)
    return in_maps


def _gather(res):
    y = np.empty((B, T, C), np.float32)
    for c in range(NCORES):
        bidx = c // 4
        r0 = (c % 4) * ROWS
        y[bidx, r0:r0 + ROWS] = res.results[c]["out"]
    return y


def kernel(**inputs):
    in_maps = _prepare_in_maps(**inputs)
    nc = _get_program()
    res = run_bass_kernel_spmd(nc, in_maps, core_ids=list(range(NCORES)))
    return _gather(res)


def run_traced(inputs, **kw):
    """Run with NTFF tracing; returns (output, BassKernelResults)."""
    in_maps = _prepare_in_maps(**inputs)
    nc = _get_program()
    res = run_bass_kernel_spmd(nc, in_maps, core_ids=list(range(NCORES)),
                               trace=True, **kw)
    return _gather(res), res


# revision 25
# speedup vs baseline: 1.0085x; 1.0060x over previous
"""Trainium2 Bass kernel for a dense transformer block (B=2, T=2048, C=1024, H=16).

Sharding (v2, tensor-parallel attention + ReduceScatter):
  core c -> batch b = c//4, head-group g = c%4 (heads 4g..4g+3),
  own token rows r0 = 512*g of its batch.

Per core:
  LN1 over the full sequence (token-major) -> PE-transpose -> xnT
  (feature-major).  qkv only for the core's 4 heads (q,k feature-major
  via pattern B; v token-major via pattern A with a ones column per head
  for the softmax denominator).  Attention is block-causal EXACT and
  static: every core processes all 2048 queries for its heads, so the
  causal structure is identical across cores; only 4 static staircase
  masks are needed on the diagonal band.  Scores S^T = k^T.T @ q^T (keys
  on partitions), exp on ACT (scale=1/8 folded in), av accumulates the
  denominator in row 64, normalize via reciprocal + gpsimd
  partition_broadcast.  proj partial = y_heads @ Wp[head rows] computed
  token-major (pattern A), then ReduceScatter(add) over the 4-core batch
  group hands each core the reduced rows it owns.  Residual + LN2 + MLP
  (row-sharded, fc/fc2 full) finish locally.

LayerNorm affines are folded into the following matmul weights on the
host (exact).  Matmuls run as float32r (full PE rate at N>=256);
attention q/k/v/P are bf16.
"""

from contextlib import ExitStack

import ml_dtypes
import numpy as np

import concourse.bass as bass
import concourse.tile as tile
import concourse.bacc as bacc
import concourse.mybir as mybir
from concourse.bass_utils import run_bass_kernel_spmd
from concourse.masks import make_identity

F32 = mybir.dt.float32
F32R = mybir.dt.float32r
BF16 = mybir.dt.bfloat16
ALU = mybir.AluOpType
ACTF = mybir.ActivationFunctionType

B, T, C = 2, 2048, 1024
H, DH = 16, 64
FF = 4096
EPS = 1e-5
NCORES = 8
ROWS = 512            # token rows owned per core (MLP phase)
HG = 4                # heads per core
HGF = HG * DH         # 256 head-group features
NTT = T // 128        # 16 token tiles
NOT = ROWS // 128     # 4 own token tiles
NCP = C // 128        # 8 feature partition-tiles of C
NFP = FF // 128       # 32 feature partition-tiles of FF
NQC = T // 512        # 4 query chunks
VSTRIDE = DH + 1      # v stored with a ones column per head


def r(ap, pat, **kw):
    return ap.rearrange(pat, **kw)


def build_program():
    nc = bacc.Bacc("TRN2", target_bir_lowering=False, debug=False,
                   num_devices=NCORES)

    def din(name, shape, dtype=F32):
        return nc.dram_tensor(name, list(shape), dtype, kind="ExternalInput")

    xb = din("xb", (T, C))
    xo_d = din("xo", (ROWS, C))
    cmask = din("cmask", (128, 4 * 512), BF16)
    w_qkv = din("w_qkv", (C, 3 * HGF), BF16)      # q|k|v for this head group
    w_pr = din("w_pr", (HGF, C), F32R)            # proj rows for this group
    w_fc = din("w_fc", (C, FF), F32R)
    w_fc2 = din("w_fc2", (FF, C), F32R)
    b_qk_col = din("b_qk_col", (128, 4))          # q0 q1 k0 k1 bias columns
    b_v_bc = din("b_v_bc", (128, HGF))
    b_proj_bc = din("b_proj_bc", (128, C))
    b_fc_col = din("b_fc_col", (128, 32))
    b_fc2_col = din("b_fc2_col", (128, 8))
    out = nc.dram_tensor("out", [ROWS, C], F32, kind="ExternalOutput")

    with tile.TileContext(nc) as tc, ExitStack() as ctx:
        # ---- constants ----
        cpool = ctx.enter_context(tc.tile_pool(name="const", bufs=1))
        ident = cpool.tile([128, 128], F32, tag="ident")
        make_identity(nc, ident[:])
        bqk = cpool.tile([128, 4], F32, tag="bqk")
        bvbc = cpool.tile([128, HGF], F32, tag="bvbc")
        bprbc = cpool.tile([128, C], F32, tag="bprbc")
        bfc = cpool.tile([128, 32], F32, tag="bfc")
        bfc2 = cpool.tile([128, 8], F32, tag="bfc2")
        mtile = cpool.tile([128, 4 * 512], BF16, tag="mtile")
        epsc = cpool.tile([128, 1], F32, tag="epsc")
        nc.gpsimd.memset(epsc[:], EPS)

        def load_consts():
            # deferred so these DMAs queue behind the critical first x tiles
            nc.sync.dma_start(bqk[:], b_qk_col.ap())
            nc.sync.dma_start(bvbc[:], b_v_bc.ap())
            nc.sync.dma_start(bprbc[:], b_proj_bc.ap())
            nc.sync.dma_start(bfc[:], b_fc_col.ap())
            nc.sync.dma_start(bfc2[:], b_fc2_col.ap())
            nc.sync.dma_start(mtile[:], cmask.ap())

        def layernorm_apply(spool, xt, xn_out, on_act=False):
            """xn_out = (xt - mean) * rsqrt(var + eps), rowwise over 1024."""
            st = spool.tile([128, 12], F32, tag="st")
            nc.vector.bn_stats(st[:, 0:6], xt[:, 0:512])
            nc.vector.bn_stats(st[:, 6:12], xt[:, 512:1024])
            ag = spool.tile([128, 2], F32, tag="ag")
            nc.vector.bn_aggr(ag[:], r(st, "p (c s) -> p c s", s=6))
            sd = spool.tile([128, 1], F32, tag="sd")
            nc.scalar.activation(sd[:], ag[:, 1:2], ACTF.Sqrt, bias=epsc[:],
                                 scale=1.0)
            rc = spool.tile([128, 1], F32, tag="rc")
            nc.vector.reciprocal(rc[:], sd[:])
            if on_act:
                # xn = x * r + (-mu * r), as one ACT pass (DVE stays free)
                nmr = spool.tile([128, 1], F32, tag="nmr")
                nc.vector.tensor_scalar(nmr[:], ag[:, 0:1], rc[:], -1.0,
                                        op0=ALU.mult, op1=ALU.mult)
                nc.scalar.activation(xn_out, xt, ACTF.Identity,
                                     bias=nmr[:], scale=rc[:])
            else:
                nc.vector.tensor_scalar(xn_out, xt, ag[:, 0:1], rc[:],
                                        op0=ALU.subtract, op1=ALU.mult)

        # yT: attention output, feature-major [2 x (2 heads)][128, T]
        yTpool = ctx.enter_context(tc.tile_pool(name="yT", bufs=1))
        yT = [yTpool.tile([128, T], F32R, tag=f"yT{i}", name=f"yT{i}")
              for i in range(2)]
        # DRAM bounce buffers for the collective
        drpool = ctx.enter_context(tc.tile_pool(name="dram", bufs=1,
                                                space="DRAM"))
        pp_d = drpool.tile([T, C], BF16, tag="pp_d", name="pp_d")
        rs_d = drpool.tile([ROWS, C], BF16, tag="rs_d", name="rs_d")

        with ExitStack() as qctx:
            kvq = qctx.enter_context(tc.tile_pool(name="kvq", bufs=1))
            qTb = [kvq.tile([128, T], BF16, tag=f"qT{i}", name=f"qT{i}")
                   for i in range(2)]
            kTb = [kvq.tile([128, T], BF16, tag=f"kT{i}", name=f"kT{i}")
                   for i in range(2)]
            vb = [kvq.tile([128, HG * VSTRIDE], BF16, tag=f"v{i}", name=f"v{i}")
                  for i in range(NTT)]

            # ============= Phase A: LN1 + transpose to feature-major =======
            with tc.tile_pool(name="phAxnT", bufs=1) as xnTpool:
                xnT = [xnTpool.tile([128, T], BF16, tag=f"xnT{i}",
                                    name=f"xnT{i}") for i in range(NCP)]
                with tc.tile_pool(name="phA", bufs=4) as apool, \
                     tc.tile_pool(name="phAs", bufs=6) as aspool, \
                     tc.tile_pool(name="phAps", bufs=8, space="PSUM") as atps:
                    identb = apool.tile([128, 128], BF16, tag="identb",
                                        bufs=1)
                    nc.vector.tensor_copy(identb[:], ident[:])
                    for tt in range(NTT):
                        xt = apool.tile([128, C], F32, tag="x")
                        nc.sync.dma_start(xt[:],
                                          xb.ap()[tt * 128:(tt + 1) * 128, :])
                        if tt == 1:
                            load_consts()
                        xn = apool.tile([128, C], BF16, tag="xn")
                        layernorm_apply(aspool, xt[:], xn[:], on_act=True)
                        for pt in range(NCP):
                            tp = atps.tile([128, 128], BF16, tag="tp",
                                           name=f"tp_{tt}_{pt}")
                            nc.tensor.transpose(
                                tp[:], xn[:, pt * 128:(pt + 1) * 128],
                                identb[:])
                            if pt < 2:
                                nc.scalar.copy(
                                    xnT[pt][:, tt * 128:(tt + 1) * 128], tp[:])
                            else:
                                nc.vector.tensor_copy(
                                    xnT[pt][:, tt * 128:(tt + 1) * 128], tp[:])

                # ============= Phase B: qkv for this head group ============
                with tc.tile_pool(name="wqkv", bufs=1) as wpool, \
                     tc.tile_pool(name="qkps", bufs=3, space="PSUM") as qkps:
                    wq = [wpool.tile([128, 3 * HGF], BF16, tag=f"wq{i}",
                                     name=f"wq{i}") for i in range(NCP)]
                    for kt in range(NCP):
                        nc.sync.dma_start(
                            wq[kt][:], w_qkv.ap()[kt * 128:(kt + 1) * 128, :])
                    # q,k feature-major (pattern B): M-tiles q0 q1 k0 k1
                    for m in range(4):
                        dstl = qTb if m < 2 else kTb
                        dst = dstl[m % 2]
                        for tcix in range(NQC):
                            ps = qkps.tile([128, 512], F32, tag="qk")
                            for kt in range(NCP):
                                nc.tensor.matmul(
                                    ps[:],
                                    wq[kt][:, m * 128:(m + 1) * 128],
                                    xnT[kt][:, tcix * 512:(tcix + 1) * 512],
                                    start=(kt == 0), stop=(kt == NCP - 1))
                            nc.vector.tensor_scalar(
                                dst[:, tcix * 512:(tcix + 1) * 512], ps[:],
                                bqk[:, m:m + 1], None, op0=ALU.add)
                    # v token-major (pattern A) with ones column
                    for tt in range(NTT):
                        nc.gpsimd.memset(
                            r(vb[tt], "p (h m) -> p h m",
                              m=VSTRIDE)[:, :, DH:DH + 1], 1.0)
                        ps = qkps.tile([128, HGF], F32, tag="vp")
                        for kt in range(NCP):
                            nc.tensor.matmul(
                                ps[:],
                                xnT[kt][:, tt * 128:(tt + 1) * 128],
                                wq[kt][:, 2 * HGF:3 * HGF],
                                start=(kt == 0), stop=(kt == NCP - 1))
                        dst = r(vb[tt], "p (h m) -> p h m",
                                m=VSTRIDE)[:, :, 0:DH]
                        nc.vector.tensor_tensor(
                            dst, r(ps[:], "p (h m) -> p h m", m=DH),
                            r(bvbc[:], "p (h m) -> p h m", m=DH), op=ALU.add)

            # ============= Phase C: attention (exact block-causal) =========
            with tc.tile_pool(name="att", bufs=4) as atpool, \
                 tc.tile_pool(name="attsm", bufs=3) as smpool, \
                 tc.tile_pool(name="scps", bufs=2, space="PSUM") as scps, \
                 tc.tile_pool(name="avps", bufs=1, space="PSUM") as avps:
                for pt in range(2):
                    for qc in range(NQC):
                        nkt = 4 * (qc + 1)
                        avs = [avps.tile([128, 512], F32,
                                         tag=f"av{qc % 2}{s}",
                                         name=f"av_{pt}_{qc}_{s}")
                               for s in range(2)]
                        for kp in range(nkt // 2):
                            for sub in range(2):
                                h = 2 * pt + sub
                                hb = 64 * sub
                                sc = scps.tile([128, 1024], F32, tag="sc")
                                for j in range(2):
                                    kt = 2 * kp + j
                                    nc.tensor.matmul(
                                        sc[:, j * 512:(j + 1) * 512],
                                        kTb[pt][hb:hb + 64,
                                                kt * 128:(kt + 1) * 128],
                                        qTb[pt][hb:hb + 64,
                                                qc * 512:(qc + 1) * 512],
                                        start=True, stop=True)
                                et = atpool.tile([128, 1024], BF16, tag="e")
                                nc.scalar.activation(et[:], sc[:], ACTF.Exp,
                                                     scale=0.125)
                                for j in range(2):
                                    kt = 2 * kp + j
                                    band = kt - 4 * qc
                                    if band >= 0:
                                        pm = atpool.tile([128, 512], BF16,
                                                         tag="p")
                                        nc.vector.tensor_tensor(
                                            pm[:],
                                            et[:, j * 512:(j + 1) * 512],
                                            mtile[:,
                                                  band * 512:(band + 1) * 512],
                                            op=ALU.mult)
                                        rhs_av = pm[:]
                                    else:
                                        rhs_av = et[:, j * 512:(j + 1) * 512]
                                    nc.tensor.matmul(
                                        avs[sub][0:VSTRIDE, :],
                                        vb[kt][:,
                                               h * VSTRIDE:(h + 1) * VSTRIDE],
                                        rhs_av,
                                        start=(kt == 0), stop=(kt == nkt - 1),
                                        skip_group_check=True)
                        for sub in range(2):
                            hb = 64 * sub
                            rr = smpool.tile([1, 512], F32, tag="rr")
                            nc.vector.reciprocal(rr[:],
                                                 avs[sub][DH:DH + 1, :])
                            bc = smpool.tile([64, 512], F32, tag="bc")
                            nc.gpsimd.partition_broadcast(bc[:], rr[:])
                            nc.vector.tensor_tensor(
                                yT[pt][hb:hb + 64, qc * 512:(qc + 1) * 512],
                                avs[sub][0:DH, :], bc[:], op=ALU.mult)

        # ============= Phase D: proj partial (token-major) + RS ============
        with tc.tile_pool(name="phDw", bufs=1) as dwpool, \
             tc.tile_pool(name="phDe", bufs=3) as depool, \
             tc.tile_pool(name="pps", bufs=3, space="PSUM") as pps:
            wp = [dwpool.tile([128, C], F32R, tag=f"wp{i}", name=f"wp{i}")
                  for i in range(2)]
            for i in range(2):
                nc.sync.dma_start(wp[i][:], w_pr.ap()[i * 128:(i + 1) * 128, :])
            for tt in range(NTT):
                pe = depool.tile([128, C], BF16, tag="pe")
                for cc in range(2):
                    ps = pps.tile([128, 512], F32, tag="pj")
                    for i in range(2):
                        nc.tensor.matmul(
                            ps[:], yT[i][:, tt * 128:(tt + 1) * 128],
                            wp[i][:, cc * 512:(cc + 1) * 512],
                            start=(i == 0), stop=(i == 1))
                    nc.vector.tensor_copy(pe[:, cc * 512:(cc + 1) * 512], ps[:])
                nc.sync.dma_start(pp_d[tt * 128:(tt + 1) * 128, :], pe[:])
            nc.gpsimd.collective_compute(
                "ReduceScatter", ALU.add,
                replica_groups=[[0, 1, 2, 3], [4, 5, 6, 7]],
                ins=[pp_d.opt()], outs=[rs_d.opt()])

        # ============= Phase E: residual + LN2 =============================
        dxpool = ctx.enter_context(tc.tile_pool(name="phDx", bufs=1))
        x2 = [dxpool.tile([128, C], F32, tag=f"x2{i}", name=f"x2{i}")
              for i in range(NOT)]
        xn2T = [dxpool.tile([128, ROWS], F32R, tag=f"xn2T{i}", name=f"xn2T{i}")
                for i in range(NCP)]
        with tc.tile_pool(name="phE", bufs=3) as epool, \
             tc.tile_pool(name="phEs", bufs=4) as espool, \
             tc.tile_pool(name="tps", bufs=4, space="PSUM") as tps:
            for j in range(NOT):
                rs_sb = epool.tile([128, C], BF16, tag="rs")
                nc.sync.dma_start(rs_sb[:], rs_d[j * 128:(j + 1) * 128, :])
                xot = epool.tile([128, C], F32, tag="xot")
                nc.sync.dma_start(xot[:], xo_d.ap()[j * 128:(j + 1) * 128, :])
                xr = epool.tile([128, C], F32, tag="xr")
                nc.vector.tensor_tensor(xr[:], rs_sb[:], bprbc[:], op=ALU.add)
                nc.vector.tensor_tensor(x2[j][:], xr[:], xot[:], op=ALU.add)
                xn2 = epool.tile([128, C], F32, tag="xn2")
                layernorm_apply(espool, x2[j][:], xn2[:], on_act=True)
                for pt in range(NCP):
                    tp = tps.tile([128, 128], F32, tag="tp")
                    nc.tensor.transpose(tp[:], xn2[:, pt * 128:(pt + 1) * 128],
                                        ident[:])
                    nc.vector.tensor_copy(xn2T[pt][:, j * 128:(j + 1) * 128],
                                          tp[:])

        # ============= Phase F: MLP + out ==================================
        with tc.tile_pool(name="phF", bufs=2) as fpool, \
             tc.tile_pool(name="phFh", bufs=1) as hpool, \
             tc.tile_pool(name="fps", bufs=2, space="PSUM") as fps, \
             tc.tile_pool(name="tps2", bufs=4, space="PSUM") as tps2:
            hgT = [hpool.tile([128, ROWS], F32R, tag=f"hg{i}", name=f"hg{i}")
                   for i in range(NFP)]
            outsb = [hpool.tile([128, C], F32, tag=f"os{i}", name=f"os{i}")
                     for i in range(NOT)]
            for m in range(NFP):
                wt = fpool.tile([128, C], F32R, tag="wf")
                nc.sync.dma_start(
                    r(wt[:], "p (k c) -> p k c", c=128),
                    r(w_fc.ap()[:, m * 128:(m + 1) * 128],
                      "(k p) c -> p k c", p=128))
                ps = fps.tile([128, 512], F32, tag="fc")
                for kt in range(NCP):
                    nc.tensor.matmul(
                        ps[:], wt[:, kt * 128:(kt + 1) * 128], xn2T[kt][:],
                        start=(kt == 0), stop=(kt == NCP - 1))
                nc.scalar.activation(hgT[m][:], ps[:], ACTF.Gelu,
                                     bias=bfc[:, m:m + 1], scale=1.0)
            for m2 in range(8):
                wt = fpool.tile([128, FF], F32R, tag="wf2", bufs=2)
                nc.sync.dma_start(
                    r(wt[:], "p (k c) -> p k c", c=128),
                    r(w_fc2.ap()[:, m2 * 128:(m2 + 1) * 128],
                      "(k p) c -> p k c", p=128))
                ps = fps.tile([128, 512], F32, tag="fc")
                for kt2 in range(NFP):
                    nc.tensor.matmul(
                        ps[:], wt[:, kt2 * 128:(kt2 + 1) * 128], hgT[kt2][:],
                        start=(kt2 == 0), stop=(kt2 == NFP - 1))
                y2 = fpool.tile([128, 512], F32, tag="y2")
                nc.vector.tensor_scalar(y2[:], ps[:], bfc2[:, m2:m2 + 1], None,
                                        op0=ALU.add)
                for j in range(NOT):
                    tp = tps2.tile([128, 128], F32, tag="tp")
                    nc.tensor.transpose(tp[:], y2[:, j * 128:(j + 1) * 128],
                                        ident[:])
                    nc.vector.tensor_tensor(
                        outsb[j][:, m2 * 128:(m2 + 1) * 128], tp[:],
                        x2[j][:, m2 * 128:(m2 + 1) * 128], op=ALU.add)
            for j in range(NOT):
                nc.sync.dma_start(out.ap()[j * 128:(j + 1) * 128, :],
                                  outsb[j][:])

    nc.compile()
    return nc


_NC_CACHE = None


def _get_program():
    global _NC_CACHE
    if _NC_CACHE is None:
        _NC_CACHE = build_program()
    return _NC_CACHE


def _prepare_in_maps(x, ln1_g, ln1_b, w_attn, b_attn, w_proj, b_proj,
                     ln2_g, ln2_b, w_fc, b_fc, w_fc2, b_fc2):
    x = np.asarray(x, np.float32)
    ln1_g = np.asarray(ln1_g, np.float32); ln1_b = np.asarray(ln1_b, np.float32)
    w_attn = np.asarray(w_attn, np.float32); b_attn = np.asarray(b_attn, np.float32)
    w_proj = np.asarray(w_proj, np.float32); b_proj = np.asarray(b_proj, np.float32)
    ln2_g = np.asarray(ln2_g, np.float32); ln2_b = np.asarray(ln2_b, np.float32)
    w_fc = np.asarray(w_fc, np.float32); b_fc = np.asarray(b_fc, np.float32)
    w_fc2 = np.asarray(w_fc2, np.float32); b_fc2 = np.asarray(b_fc2, np.float32)

    # Fold LayerNorm affine params into the following matmuls (exact).
    w_attn_f = ln1_g[:, None] * w_attn
    b_attn_f = b_attn + ln1_b @ w_attn
    w_fc_f = ln2_g[:, None] * w_fc
    b_fc_f = b_fc + ln2_b @ w_fc

    # 4 static staircase masks for the diagonal band:
    # mask_i[p, j] = 1 iff j >= 128*i + p (query col j attends band-tile row p)
    jj = np.arange(512)[None, :]
    pp = np.arange(128)[:, None]
    cmask = np.concatenate(
        [(jj >= 128 * i + pp) for i in range(4)],
        axis=1).astype(ml_dtypes.bfloat16)

    shared = {
        "cmask": cmask,
        "w_fc": w_fc_f,
        "w_fc2": w_fc2,
        "b_proj_bc": np.ascontiguousarray(np.broadcast_to(b_proj, (128, C))),
        "b_fc_col": np.ascontiguousarray(b_fc_f.reshape(32, 128).T),
        "b_fc2_col": np.ascontiguousarray(b_fc2.reshape(8, 128).T),
    }

    in_maps = []
    for c in range(NCORES):
        bidx = c // 4
        g = c % 4
        r0 = g * ROWS
        fsl = slice(g * HGF, (g + 1) * HGF)
        w_q = w_attn_f[:, 0 * C:1 * C][:, fsl]
        w_k = w_attn_f[:, 1 * C:2 * C][:, fsl]
        w_v = w_attn_f[:, 2 * C:3 * C][:, fsl]
        b_q = b_attn_f[0 * C:1 * C][fsl]
        b_k = b_attn_f[1 * C:2 * C][fsl]
        b_v = b_attn_f[2 * C:3 * C][fsl]
        m = dict(shared)
        m["xb"] = np.ascontiguousarray(x[bidx])
        m["xo"] = np.ascontiguousarray(x[bidx][r0:r0 + ROWS])
        m["w_qkv"] = np.ascontiguousarray(
            np.concatenate([w_q, w_k, w_v], axis=1)).astype(ml_dtypes.bfloat16)
        m["w_pr"] = np.ascontiguousarray(w_proj[fsl, :])
        m["b_qk_col"] = np.ascontiguousarray(
            np.concatenate([b_q, b_k]).reshape(4, 128).T)
        m["b_v_bc"] = np.ascontiguousarray(np.broadcast_to(b_v, (128, HGF)))
        in_maps.append(m)
    return in_maps


def _gather(res):
    y = np.empty((B, T, C), np.float32)
    for c in range(NCORES):
        bidx = c // 4
        r0 = (c % 4) * ROWS
        y[bidx, r0:r0 + ROWS] = res.results[c]["out"]
    return y


def kernel(**inputs):
    in_maps = _prepare_in_maps(**inputs)
    nc = _get_program()
    res = run_bass_kernel_spmd(nc, in_maps, core_ids=list(range(NCORES)))
    return _gather(res)


def run_traced(inputs, **kw):
    """Run with NTFF tracing; returns (output, BassKernelResults)."""
    in_maps = _prepare_in_maps(**inputs)
    nc = _get_program()
    res = run_bass_kernel_spmd(nc, in_maps, core_ids=list(range(NCORES)),
                               trace=True, **kw)
    return _gather(res), res
